# revision 16
# baseline (speedup 1.0000x reference)
"""RNN-T JointNetwork kernel for 8 Trainium2 NeuronCores.

Math: out[b,t,u,:] = tanh(concat(fe[b,t], gd[b,u])) @ Wj + bj
with fe = f@We+be, gd = g@Wd+bd.

Since tanh acts elementwise and the concat feeds a single GEMM, the joint
GEMM factorizes exactly:
    out[b,t,u,:] = A[b,t,:] + C[b,u,:]
    A = tanh(f@We+be) @ Wj[:Dm]          (per-(b,t) row)
    C = tanh(g@Wd+bd) @ Wj[Dm:] + bj     (per-(b,u) row)
This collapses the 137-GFLOP joint GEMM into two tiny GEMMs plus a
broadcast-add, leaving the kernel bound by the output write.

Sharding: 8 cores, core c owns (b = c//2, t-half = c%2) -> a [128,64,V]
output chunk per core.

Measured facts this design is built around (profiled on trn2):
  - each dma_start costs ~650 ns serial issue time on the Sync sequencer
    -> pack inputs into 9 DMAs
  - concurrent DMA queues interleave at packet level, so an issue-order
    "pipeline" does NOT give early completion of the first tensor ->
    explicit add_dep_helper edges serialize the input stream into stages
    (pack1 -> Wj -> selectors) so each dependency lands ASAP
  - gpsimd affine_select is ~5 us per op -> selector constants are
    shipped from the host inside the input stream
  - DMA write bandwidth ~400 GB/s only with large contiguous runs ->
    output rows are permuted so each partition writes 4 consecutive
    DRAM rows = one 8 KB descriptor (out tensor is bf16: tolerance 2e-2
    dwarfs bf16's ~5e-3; host upcasts to fp32)
  - fp32 PSUM->SBUF moves run at 1x on DVE and ACT alike -> alternate
    the per-tile output moves across both engines
  - PE HAM clock gate starts at 1.2 GHz; ~15 dummy matmuls during the
    input DMA window warm it to 2.4 GHz before the real GEMMs

On-core plan (bf16 everywhere, fp32 only in PSUM):
  - tfT[m,t] = tanh(We.T@fT + be), tgT likewise (PE bf16 + ACT tanh)
  - ACp0 = [A(0:64) ; C] and ACp1 = [C ; A(64:128)] packed bf16 [128,V];
    C is computed once into psum rows 64:128 (+bj via K=1 ones matmul),
    copied to ACp0, then replicated to ACp1's rows 0:64 with a K=64
    identity matmul (cheaper than re-running the K=512 GEMM)
  - output chunk j covers out rows 512j..512j+512; psum tile a holds
    rows 4p+a so partition p's SBUF bytes map to 4 consecutive DRAM
    rows; ONE K=128 selector matmul per 512-col bank picks the A row
    and C row and sums them in fp32 PSUM
"""

import sys

sys.path.insert(0, "/opt/trn_rl_repo")

import numpy as np

import concourse.bacc as bacc
import concourse.mybir as mybir
import concourse.tile as tile
from concourse.bass_utils import run_bass_kernel_spmd
from concourse.tile_rust import add_dep_helper

B, T, U = 4, 256, 64
D = 512  # DE = DD = DM
V = 1024
TC = 128  # t rows per core
NCORES = 8
FP32 = mybir.dt.float32
BF16 = mybir.dt.bfloat16
NPBF16 = mybir.dt.np(mybir.dt.bfloat16)
FP8 = mybir.dt.float8e4
NPFP8 = mybir.dt.np(mybir.dt.float8e4)
TANH = mybir.ActivationFunctionType.Tanh

# pack1 column offsets (per-core tensor: fT | We | gT | Wd | identity)
OFF_FT, OFF_WE, OFF_GT, OFF_WD, OFF_ID = 0, 512, 2560, 2816, 4864
PACK1_COLS = 4992
# bjp row-tensor offsets: bj | ones(128) | be row | bd row
OFF_ONES, OFF_BE, OFF_BD = V, V + 128, V + 640
BJP_COLS = V + 1152

_cache = {}


def _build_nc():
    nc = bacc.Bacc("TRN2", target_bir_lowering=False)

    pack1_d = nc.dram_tensor("pack1", [128, PACK1_COLS], BF16, kind="ExternalInput")
    wjt_d = [nc.dram_tensor(f"wjt{i}", [128, 2048], BF16, kind="ExternalInput") for i in range(2)]
    wjb_d = [nc.dram_tensor(f"wjb{i}", [128, 2048], BF16, kind="ExternalInput") for i in range(2)]
    sel_d = [nc.dram_tensor(f"sel{i}", [128, 4096], FP8, kind="ExternalInput") for i in range(2)]
    bpack_d = nc.dram_tensor("bpack", [128, 8], FP32, kind="ExternalInput")
    bjp_d = nc.dram_tensor("bjp", [1, BJP_COLS], BF16, kind="ExternalInput")
    out_d = nc.dram_tensor("out", [16 * 128, 4 * V], BF16, kind="ExternalOutput")

    with tile.TileContext(nc) as tc:
        with tc.tile_pool(name="wts", bufs=1) as wp:
            pack1 = wp.tile([128, PACK1_COLS], BF16, tag="pack1")
            wjt = [wp.tile([128, 2048], BF16, tag=f"wjt{i}", name=f"wjt{i}") for i in range(2)]
            wjb = [wp.tile([128, 2048], BF16, tag=f"wjb{i}", name=f"wjb{i}") for i in range(2)]
            sel = [wp.tile([128, 4096], FP8, tag=f"sel{i}", name=f"sel{i}") for i in range(2)]
            bpack = wp.tile([128, 8], FP32, tag="bpack")
            bjp = wp.tile([1, BJP_COLS], BF16, tag="bjp")
            tfT = [wp.tile([128, TC], BF16, tag=f"tfT{c}", name=f"tfT{c}") for c in range(4)]
            tgT = [wp.tile([128, U], BF16, tag=f"tgT{c}", name=f"tgT{c}") for c in range(4)]
            ACp0 = wp.tile([128, V], BF16, tag="ACp0")
            ACp1 = wp.tile([128, V], BF16, tag="ACp1")

            # input stream: issued back-to-back; transfers interleave at
            # packet level across queues (measured: concurrent DMAs
            # aggregate to ~400 GB/s, a lone DMA only ~270), so no
            # explicit ordering -- everything lands by ~22 us and the
            # prologue GEMMs interleave with the arrivals
            nc.sync.dma_start(pack1[:], pack1_d[:])
            nc.sync.dma_start(bpack[:], bpack_d[:])
            nc.sync.dma_start(bjp[:], bjp_d[:])
            for i in range(2):
                nc.sync.dma_start(wjb[i][:], wjb_d[i][:])
            for i in range(2):
                nc.sync.dma_start(wjt[i][:], wjt_d[i][:])
            nc.sync.dma_start(sel[0][:], sel_d[0][:])
            nc.sync.dma_start(sel[1][:], sel_d[1][:])

            # views into pack1
            fT = [pack1[:, OFF_FT + c * 128 : OFF_FT + (c + 1) * 128] for c in range(4)]
            We = [pack1[:, OFF_WE + c * 512 : OFF_WE + (c + 1) * 512] for c in range(4)]
            gT = [pack1[:, OFF_GT + c * 64 : OFF_GT + (c + 1) * 64] for c in range(4)]
            Wd = [pack1[:, OFF_WD + c * 512 : OFF_WD + (c + 1) * 512] for c in range(4)]
            # wj chunk mc, v-half vh  ->  tile mc//2, cols (mc%2)*1024 + vh*512
            wj_t = lambda mc, vh: wjt[mc // 2][:, (mc % 2) * 1024 + vh * 512 : (mc % 2) * 1024 + vh * 512 + 512]
            wj_b = lambda mc, vh: wjb[mc // 2][:, (mc % 2) * 1024 + vh * 512 : (mc % 2) * 1024 + vh * 512 + 512]

            # ---- prologue ----
            with tc.tile_pool(name="pp", bufs=4, space="PSUM") as pp:
                # PE warm-up (see module doc); results never read
                scratch = wp.tile([128, 640], BF16, tag="scratch")
                nc.vector.memset(scratch[:], 1.0)
                wps = pp.tile([128, 512], FP32, tag="pps")
                for _ in range(18):
                    nc.tensor.matmul(
                        wps[:], scratch[:, 0:128], scratch[:, 128:640],
                        start=True, stop=True,
                    )

                # encoder/decoder projections as 5 wide matmuls each
                # (activations on partitions, bias via a K=1 ones-row
                # matmul since ACT bias is per-partition only), tanh in
                # one ACT op, then PE transposes into the [m, t] layout
                # the A/C GEMMs contract over.  g-path first: C's chain
                # is longer than A's.
                ident = pack1[:, OFF_ID : OFF_ID + 128]
                tg = wp.tile([U, D], BF16, tag="tg")
                th = wp.tile([TC, D], BF16, tag="th")
                ps = pp.tile([64, 512], FP32, tag="pps")
                for dc in range(4):
                    nc.tensor.matmul(
                        ps[:], gT[dc], Wd[dc],
                        start=(dc == 0), stop=False,
                    )
                nc.tensor.matmul(
                    ps[:], bjp[:, OFF_ONES : OFF_ONES + 64],
                    bjp[:, OFF_BD : OFF_BD + 512],
                    start=False, stop=True,
                )
                nc.scalar.activation(tg[:], ps[:], TANH)
                for mc in range(4):
                    pt = pp.tile([128, U], BF16, tag="pps")
                    nc.tensor.transpose(
                        pt[:], tg[:, mc * 128 : (mc + 1) * 128], ident[0:64, 0:64]
                    )
                    nc.vector.tensor_copy(tgT[mc][:], pt[:])
                psf = pp.tile([128, 512], FP32, tag="pps")
                for dc in range(4):
                    nc.tensor.matmul(
                        psf[:], fT[dc], We[dc],
                        start=(dc == 0), stop=False,
                    )
                nc.tensor.matmul(
                    psf[:], bjp[:, OFF_ONES : OFF_ONES + 128],
                    bjp[:, OFF_BE : OFF_BE + 512],
                    start=False, stop=True,
                )
                nc.scalar.activation(th[:], psf[:], TANH)
                for mc in range(4):
                    pt = pp.tile([128, TC], BF16, tag="pps")
                    nc.tensor.transpose(
                        pt[:], th[:, mc * 128 : (mc + 1) * 128], ident
                    )
                    nc.vector.tensor_copy(tfT[mc][:], pt[:])

                # C once into psum rows 64:128, then swap-replicate
                for vh in range(2):
                    vs = slice(vh * 512, (vh + 1) * 512)
                    ps = pp.tile([128, 512], FP32, tag="pps")
                    for mc in range(4):
                        nc.tensor.matmul(
                            ps[64:128, :], tgT[mc][:], wj_b(mc, vh),
                            start=(mc == 0), stop=False,
                        )
                    nc.tensor.matmul(
                        ps[64:128, :], bjp[:, OFF_ONES : OFF_ONES + 64], bjp[:, vs],
                        start=False, stop=True,
                    )
                    nc.scalar.copy(ACp0[64:128, vs], ps[64:128, :])
                    ps2 = pp.tile([128, 512], FP32, tag="pps")
                    nc.tensor.matmul(
                        ps2[0:64, :],
                        pack1[64:128, OFF_ID + 64 : OFF_ID + 128],
                        ACp0[64:128, vs],
                        start=True, stop=True,
                    )
                    nc.vector.tensor_copy(ACp1[0:64, vs], ps2[0:64, :])

                # A = tfT.T @ Wj_top -> ACp0[0:64], ACp1[64:128]
                for vh in range(2):
                    vs = slice(vh * 512, (vh + 1) * 512)
                    ps = pp.tile([128, 512], FP32, tag="pps")
                    for mc in range(4):
                        nc.tensor.matmul(
                            ps[:], tfT[mc][:], wj_t(mc, vh),
                            start=(mc == 0), stop=(mc == 3),
                        )
                    nc.scalar.copy(ACp0[0:64, vs], ps[0:64, :])
                    nc.vector.tensor_copy(ACp1[64:128, vs], ps[64:128, :])

            # ---- main loop: 16 chunks of [512 rows, 1024] bf16 = 1 MB ----
            # chunk j, psum tile a: psO_a[p,:] = out row 512j + 4p + a
            #   -> t = 8j + p//16, u = 4*(p%16) + a, h = j//8
            with (
                tc.tile_pool(name="po", bufs=4, space="PSUM") as po,
                tc.tile_pool(name="ob", bufs=4) as ob,
            ):
                for j in range(16):
                    h, jj = j // 8, j % 8
                    acp = (ACp0, ACp1)[h]
                    out_sb = ob.tile([128, 4 * V], BF16, tag="out")
                    for a in range(4):
                        psO = po.tile([128, V], FP32, tag="psO")
                        c0 = 128 * (4 * jj + a)
                        for vh in range(2):
                            nc.tensor.matmul(
                                psO[:, vh * 512 : (vh + 1) * 512],
                                sel[h][:, c0 : c0 + 128],
                                acp[:, vh * 512 : (vh + 1) * 512],
                                start=True, stop=True,
                            )
                        dst = out_sb[:, a * V : (a + 1) * V]
                        if a % 2 == 0:
                            nc.scalar.copy(dst, psO[:])
                        else:
                            nc.vector.tensor_copy(dst, psO[:])
                    nc.sync.dma_start(
                        out_d[128 * j : 128 * (j + 1), :], out_sb[:]
                    )

    nc.compile()
    return nc


def _chunkcat(M):
    """[N*128, C] -> [128, N*C]: stack 128-row chunks side by side."""
    n = M.shape[0] // 128
    return np.ascontiguousarray(
        M.reshape(n, 128, M.shape[1]).transpose(1, 0, 2).reshape(128, -1)
    )


def _build_selectors():
    """Row-permuted pair selectors (see main-loop comment)."""
    sel0 = np.zeros((128, 4096), np.float32)
    sel1 = np.zeros((128, 4096), np.float32)
    p = np.arange(128)
    for jj in range(8):
        for a in range(4):
            col = 128 * (4 * jj + a) + p
            tih = 8 * jj + p // 16
            u = 4 * (p % 16) + a
            sel0[tih, col] = 1.0
            sel0[64 + u, col] = 1.0
            sel1[u, col] = 1.0
            sel1[64 + tih, col] = 1.0
    return sel0.astype(NPFP8), sel1.astype(NPFP8)


def kernel(f, g, We, be, Wd, bd, Wj, bj):
    if "nc" not in _cache:
        _cache["nc"] = _build_nc()
    nc = _cache["nc"]

    b16 = lambda x: np.asarray(x, dtype=np.float32).astype(NPBF16)
    f = np.asarray(f, dtype=np.float32)
    g = np.asarray(g, dtype=np.float32)
    Wj = np.asarray(Wj, dtype=np.float32)

    sel0, sel1 = _build_selectors()
    wjt = _chunkcat(b16(Wj[:D]))
    wjb = _chunkcat(b16(Wj[D:]))
    bjp = np.zeros((1, BJP_COLS), np.float32)
    bjp[0, :V] = np.asarray(bj, dtype=np.float32)
    bjp[0, OFF_ONES : OFF_ONES + 128] = 1.0
    bjp[0, OFF_BE : OFF_BE + 512] = np.asarray(be, dtype=np.float32)
    bjp[0, OFF_BD : OFF_BD + 512] = np.asarray(bd, dtype=np.float32)
    bpack = np.zeros((128, 8), np.float32)
    for c in range(4):
        bpack[:, c] = np.asarray(be, dtype=np.float32)[c * 128 : (c + 1) * 128]
        bpack[:, 4 + c] = np.asarray(bd, dtype=np.float32)[c * 128 : (c + 1) * 128]
    We_p = _chunkcat(b16(We))
    Wd_p = _chunkcat(b16(Wd))
    swid = np.eye(128, dtype=np.float32)

    shared = {
        "wjt0": np.ascontiguousarray(wjt[:, :2048]),
        "wjt1": np.ascontiguousarray(wjt[:, 2048:]),
        "wjb0": np.ascontiguousarray(wjb[:, :2048]),
        "wjb1": np.ascontiguousarray(wjb[:, 2048:]),
        "sel0": sel0, "sel1": sel1,
        "bpack": bpack, "bjp": b16(bjp),
    }
    in_maps = []
    for c in range(NCORES):
        b, th = c // 2, c % 2
        fTp = _chunkcat(b16(f[b, th * TC : (th + 1) * TC, :].T))
        gTp = _chunkcat(b16(g[b].T))
        pack1 = np.concatenate([fTp, We_p, gTp, Wd_p, b16(swid)], axis=1)
        in_maps.append({"pack1": np.ascontiguousarray(pack1), **shared})
    res = run_bass_kernel_spmd(nc, in_maps, list(range(NCORES)))
    kernel._last_results = res

    out = np.empty((B, T, U, V), np.float32)
    for c in range(NCORES):
        b, th = c // 2, c % 2
        out[b, th * TC : (th + 1) * TC] = (
            res.results[c]["out"].astype(np.float32).reshape(TC, U, V)
        )
    return out
